# revision 23
# baseline (speedup 1.0000x reference)
"""Trainium2 Bass kernel for nn_CorrTorch_unfold (B=1, C=32, D=32, H=W=128).

Reference math (incl. its raw-reshape scramble): with
F = k2*16384 + h2*128 + w2 and (c', k', G) = unravel(F, [32, 9, 512]),
kh' = k'//3, kw' = k'%3, h' = G//4, m4 = G%4:
  out[0,k2,d,h2,w2] = leaky_relu( sum_i x[i,d,h2,w2]
                                  * y_pad[c',d,h'+kh',32*m4+kw'+i] )
Equivalently, for n = 9c'+k': k2 = n//32, m = n%32, h2 = 4m + h'//32,
w2 = 4*(h'%32) + m4  (y_pad = y shifted one slice in depth, padded 1 in
H/W). The 32-term dot runs over x channels i paired with a contiguous
32-wide w-strip of y_pad.

v3 design (products-on-DVE + reduce-on-PE):
  Partition dim packs (d_local, i) = 4*32 = 128. DVE computes bf16
  products with the i-pairing baked into a host-interleaved y layout:
  Y_kw[(d,i), c'*520 + r*4 + m4] = y_pad[c', r, 32*m4 + kw + i].
  x stays in natural (h,w) layout, replicated into 47 "m-slots"
  (slot m = rows 4*(m%32)..+4) so the mod-32 slot walk m = n%32 becomes
  affine inside each TT. One TT per (c'-pair, kw') covers (c4, kh',
  h'*m4) = 2x3x512 free elems at 0.5 cyc/elem (2x_1p bf16 mode); a few
  TTs run on the otherwise-idle GpSimd engine to offload the DVE.
  The idle PE reduces over i: lhsT ones [128,32] sums each 32-partition
  group (depth groups duplicated 8x to fill full PSUM quadrants); 16
  matmuls fill a [128,2048] PSUM tile; ACT copies it to SBUF (bf16);
  one stride-8-partition DMA per tile extracts the 16 distinct rows.
  Blocks are processed in ascending x-slot order and loads are chunked
  so compute starts ~9us in. Leaky-relu + unscramble happen on host.

Sharding: D=32 depth slices, 4 per core across 8 cores.
"""
import numpy as np

_PROG_CACHE = {}
_RUN_OPTS = {"trace": False}
_LAST_RESULT = {}

D_LOC = 4
N_CORES = 8
C = 32
H = W = 128
MX = 40            # x m-slots (31 + 8 max walk: kw + 3*kh)
YCOLS = 32 * 130 * 4   # 16640 per kw slab
NBLK = 32          # one block per c'
PCOLS = 3 * 3 * 512    # 4608 product cols per c' tile (kw, kh, h'm4)
NMM = PCOLS // 512     # 9 matmuls per c'
TOTMM = NBLK * NMM     # 288
NTILE = TOTMM // 16    # 18 psum tiles -> out dumps

# c' processed in natural order; x slot windows [m0, m0+8], m0 = (9c')%32
X_CHUNKS = [(0, 9), (9, 18), (18, 27), (27, 36), (36, 40)]
Y_CHUNKS = [(0, 2), (2, 4), (4, 6), (6, 8), (8, 12), (12, 16),
            (16, 24), (24, 32)]


def _build_program():
    import concourse.bacc as bacc
    import concourse.mybir as mybir
    from concourse.tile import TileContext
    from bass_rust import VecI64Pair

    f32 = mybir.dt.float32
    bf16 = mybir.dt.bfloat16

    def apv(base_ap, offset, dims):
        a = base_ap.copy()
        part = list(a.ap[0])
        a.ap = VecI64Pair([part] + [list(d) for d in dims])
        a.offset = a.offset + offset
        return a

    nc = bacc.Bacc()
    x_in = nc.dram_tensor("xin", [128, MX * 512], bf16, kind="ExternalInput")
    y_in = nc.dram_tensor("yin", [3, 128, YCOLS], bf16, kind="ExternalInput")
    ones_in = nc.dram_tensor("ones", [128, 32], bf16, kind="ExternalInput")
    out = nc.dram_tensor("out", [NTILE, 16, 2048], bf16,
                         kind="ExternalOutput")

    with TileContext(nc) as tc:
        with tc.tile_pool(name="const", bufs=1) as cpool, \
             tc.tile_pool(name="p", bufs=4) as ppool, \
             tc.tile_pool(name="st", bufs=2) as spool, \
             tc.tile_pool(name="ps", bufs=2, space="PSUM") as pspool:

            ones = cpool.tile([128, 32], bf16)
            nc.sync.dma_start(ones[:], ones_in[:])

            ys = cpool.tile([128, 3 * YCOLS], bf16)   # kw-major y slabs
            xs = cpool.tile([128, MX * 512], bf16)

            def yload(ci):
                c0, c1 = Y_CHUNKS[ci]
                for kw in range(3):
                    nc.sync.dma_start(
                        ys[:, kw * YCOLS + c0 * 520:kw * YCOLS + c1 * 520],
                        y_in[kw, :, c0 * 520:c1 * 520])

            def xload(xi):
                s0, s1 = X_CHUNKS[xi]
                nc.sync.dma_start(xs[:, s0 * 512:s1 * 512],
                                  x_in[:, s0 * 512:s1 * 512])

            # load order tuned so the c' sequence's deps arrive just in time
            yload(0)
            xload(0)
            yload(1)
            xload(1)
            yload(2)
            xload(2)
            yload(3)
            xload(3)
            xload(4)
            yload(4)
            yload(5)
            yload(6)
            yload(7)

            mm = 0
            ps = None
            for cp in range(NBLK):
                P = ppool.tile([128, PCOLS], bf16)
                m0 = (9 * cp) % 32
                in0 = apv(xs[:], m0 * 512,
                          [[512, 3], [3 * 512, 3], [1, 512]])
                in1 = apv(ys[:], cp * 520,
                          [[YCOLS, 3], [4, 3], [1, 512]])
                o = apv(P[:], 0,
                        [[1536, 3], [512, 3], [1, 512]])
                nc.vector.tensor_tensor(o, in0, in1, mybir.AluOpType.mult)
                for t in range(NMM):
                    r = mm % 16
                    b, q = r // 4, r % 4
                    if r == 0:
                        ps = pspool.tile([128, 2048], f32)
                    nc.tensor.matmul(ps[32 * q:32 * (q + 1),
                                        512 * b:512 * (b + 1)], ones[:],
                                     P[:, 512 * t:512 * (t + 1)],
                                     start=True, stop=True,
                                     tile_position=(0, 32 * q))
                    if r == 15:
                        stage = spool.tile([128, 2048], bf16)
                        nc.scalar.copy(stage[:], ps[:])
                        src = stage[:].copy()
                        src.ap = VecI64Pair([[8 * 2048, 16], [1, 2048]])
                        nc.gpsimd.dma_start(out[mm // 16], src)
                    mm += 1

    nc.finalize()
    return nc


def _get_program():
    if "nc" not in _PROG_CACHE:
        _PROG_CACHE["nc"] = _build_program()
    return _PROG_CACHE["nc"]


def _out_perm():
    """col (c', kw, kh, h'm4) -> flat out idx k2*16384 + h2*128 + w2."""
    if "perm" in _PROG_CACHE:
        return _PROG_CACHE["perm"]
    cp, kw, kh, hm = np.meshgrid(
        np.arange(NBLK), np.arange(3), np.arange(3), np.arange(512),
        indexing='ij')
    hp, m4 = hm // 4, hm % 4
    n = 9 * cp + 3 * kh + kw
    k2, m = n // 32, n % 32
    h2 = 4 * m + hp // 32
    w2 = 4 * (hp % 32) + m4
    perm = (k2 * 16384 + h2 * 128 + w2).reshape(-1)
    _PROG_CACHE["perm"] = perm
    return perm


def kernel(x: np.ndarray, y: np.ndarray) -> np.ndarray:
    import ml_dtypes
    from concourse.bass_utils import run_bass_kernel_spmd

    bf = ml_dtypes.bfloat16
    x = np.ascontiguousarray(np.asarray(x, dtype=np.float32))
    y = np.ascontiguousarray(np.asarray(y, dtype=np.float32))
    B, C_, D, H_, W_ = x.shape
    assert (B, C_, D, H_, W_) == (1, 32, 32, 128, 128)

    # depth-shifted, H/W-padded y (fp32, cast after gather)
    y_sp = np.zeros((D, C_, 130, 130), np.float32)
    y_sp[1:, :, 1:129, 1:129] = y[0].transpose(1, 0, 2, 3)[:-1]
    x_d = x[0].transpose(1, 0, 2, 3)  # [d, c, h, w]

    # x slab: [d, i, m, 512] = x[i, d, 4*(m%32) + col//128, col%128]
    ms = np.arange(MX) % 32
    xq = x_d.reshape(D, C_, 32, 512)                        # d i m32 col
    xq = np.ascontiguousarray(xq[:, :, ms]).astype(bf)      # d i m col

    # y slabs: [kw, d, i, c', r, m4] = y_sp[d, c', r, 32*m4 + kw + i]
    i_ar = np.arange(32)[:, None]
    m4_ar = np.arange(4)[None, :]
    yq = np.empty((3, D, 32, 32, 130, 4), bf)
    for kw in range(3):
        w_idx = 32 * m4_ar + kw + i_ar  # [i, m4]
        g = y_sp[:, :, :, w_idx]        # d c' r i m4
        yq[kw] = g.transpose(0, 3, 1, 2, 4).astype(bf)

    ones_np = np.zeros((128, 32), bf)
    for m in range(32):
        g = m // 8
        ones_np[32 * g:32 * (g + 1), m] = 1

    nc = _get_program()
    in_maps = [
        {"xin": xq[4 * j:4 * j + 4].reshape(128, MX * 512),
         "yin": np.ascontiguousarray(
             yq[:, 4 * j:4 * j + 4].reshape(3, 128, YCOLS)),
         "ones": ones_np}
        for j in range(N_CORES)
    ]
    res = run_bass_kernel_spmd(nc, in_maps, core_ids=list(range(N_CORES)),
                               trace=_RUN_OPTS["trace"])
    _LAST_RESULT["res"] = res

    perm = _out_perm()
    slabs = []
    for j in range(N_CORES):
        a = np.asarray(res.results[j]["out"]).astype(np.float32)
        a = a.reshape(NTILE, 4, 4, 4, 512)                      # t q g b n
        dec = a.transpose(2, 0, 3, 1, 4).reshape(4, 9 * 16384)  # g, colstream
        oc = np.empty((4, 9 * 16384), np.float32)
        oc[:, perm] = dec
        slabs.append(oc.reshape(4, 9, H, W))
    o = np.concatenate(slabs, axis=0)       # [32, 9, 128, 128]
    o = o.transpose(1, 0, 2, 3)             # [9, 32, 128, 128]
    o = np.where(o >= 0, o, 0.2 * o).astype(np.float32)
    return o[None]


# revision 29
# speedup vs baseline: 1.3982x; 1.3982x over previous
"""Trainium2 Bass kernel for nn_CorrTorch_unfold (B=1, C=32, D=32, H=W=128).

Reference math (incl. its raw-reshape scramble): with
F = k2*16384 + h2*128 + w2 and (c', k', G) = unravel(F, [32, 9, 512]),
kh' = k'//3, kw' = k'%3, h' = G//4, m4 = G%4:
  out[0,k2,d,h2,w2] = leaky_relu( sum_i x[i,d,h2,w2]
                                  * y_pad[c',d,h'+kh',32*m4+kw'+i] )
Equivalently, for n = 9c'+k': k2 = n//32, m = n%32, h2 = 4m + h'//32,
w2 = 4*(h'%32) + m4  (y_pad = y shifted one slice in depth, padded 1 in
H/W). The 32-term dot runs over x channels i paired with a contiguous
32-wide w-strip of y_pad.

v3 design (products-on-DVE + reduce-on-PE):
  Partition dim packs (d_local, i) = 4*32 = 128. DVE computes bf16
  products with the i-pairing baked into a host-interleaved y layout:
  Y_kw[(d,i), c'*520 + r*4 + m4] = y_pad[c', r, 32*m4 + kw + i].
  x stays in natural (h,w) layout, replicated into 47 "m-slots"
  (slot m = rows 4*(m%32)..+4) so the mod-32 slot walk m = n%32 becomes
  affine inside each TT. One TT per (c'-pair, kw') covers (c4, kh',
  h'*m4) = 2x3x512 free elems at 0.5 cyc/elem (2x_1p bf16 mode); a few
  TTs run on the otherwise-idle GpSimd engine to offload the DVE.
  The idle PE reduces over i: lhsT ones [128,32] sums each 32-partition
  group (depth groups duplicated 8x to fill full PSUM quadrants); 16
  matmuls fill a [128,2048] PSUM tile; ACT copies it to SBUF (bf16);
  one stride-8-partition DMA per tile extracts the 16 distinct rows.
  Blocks are processed in ascending x-slot order and loads are chunked
  so compute starts ~9us in. Leaky-relu + unscramble happen on host.

Sharding: D=32 depth slices, 4 per core across 8 cores.
"""
import numpy as np

_PROG_CACHE = {}
_RUN_OPTS = {"trace": False}
_LAST_RESULT = {}

D_LOC = 4
N_CORES = 8
C = 32
H = W = 128
MX = 40            # x m-slots (31 + 8 max walk: kw + 3*kh)
YCOLS = 32 * 130 * 4   # 16640 per kw slab
NBLK = 32          # one block per c'
PCOLS = 3 * 3 * 512    # 4608 product cols per c' tile (kw, kh, h'm4)
NMM = PCOLS // 512     # 9 matmuls per c'
TOTMM = NBLK * NMM     # 288
NTILE = TOTMM // 16    # 18 psum tiles -> out dumps

# process c' so the x slot window [m0, m0+8] (m0 = 9c'%32) advances by
# one slot per step: c' = 25t mod 32 gives m0 = t. Loads then stream
# just ahead of compute (y per-c' chunks, x in 5 slot-range chunks).
CP_ORDER = [(25 * t) % 32 for t in range(NBLK)]
X_CHUNKS = [(0, 9), (9, 17), (17, 25), (25, 33), (33, 40)]


def _build_program():
    import concourse.bacc as bacc
    import concourse.mybir as mybir
    from concourse.tile import TileContext
    from bass_rust import VecI64Pair

    f32 = mybir.dt.float32
    bf16 = mybir.dt.bfloat16

    def apv(base_ap, offset, dims):
        a = base_ap.copy()
        part = list(a.ap[0])
        a.ap = VecI64Pair([part] + [list(d) for d in dims])
        a.offset = a.offset + offset
        return a

    nc = bacc.Bacc()
    x_in = nc.dram_tensor("xin", [128, MX * 512], bf16, kind="ExternalInput")
    y_in = nc.dram_tensor("yin", [128, NBLK * 1560], bf16,
                          kind="ExternalInput")
    ones_in = nc.dram_tensor("ones", [128, 32], bf16, kind="ExternalInput")
    out = nc.dram_tensor("out", [NTILE, 16, 2048], bf16,
                         kind="ExternalOutput")

    with TileContext(nc) as tc:
        with tc.tile_pool(name="const", bufs=1) as cpool, \
             tc.tile_pool(name="p", bufs=4) as ppool, \
             tc.tile_pool(name="st", bufs=2) as spool, \
             tc.tile_pool(name="ps", bufs=2, space="PSUM") as pspool:

            ones = cpool.tile([128, 32], bf16)
            nc.sync.dma_start(ones[:], ones_in[:])

            ys = cpool.tile([128, NBLK * 1560], bf16)  # kw inner per c'
            xs = cpool.tile([128, MX * 512], bf16)

            def yload(cp):
                nc.sync.dma_start(ys[:, cp * 1560:(cp + 1) * 1560],
                                  y_in[:, cp * 1560:(cp + 1) * 1560])

            def xload(xi):
                s0, s1 = X_CHUNKS[xi]
                nc.sync.dma_start(xs[:, s0 * 512:s1 * 512],
                                  x_in[:, s0 * 512:s1 * 512])

            # loads stream just ahead of the CP_ORDER compute sequence
            xload(0)
            yload(CP_ORDER[0])
            xload(1)
            for t in range(1, 8):
                yload(CP_ORDER[t])
            xload(2)
            for t in range(8, 16):
                yload(CP_ORDER[t])
            xload(3)
            for t in range(16, 24):
                yload(CP_ORDER[t])
            xload(4)
            for t in range(24, 32):
                yload(CP_ORDER[t])

            mm = 0
            ps = None
            for cp in CP_ORDER:
                P = ppool.tile([128, PCOLS], bf16)
                m0 = (9 * cp) % 32
                in0 = apv(xs[:], m0 * 512,
                          [[512, 3], [3 * 512, 3], [1, 512]])
                in1 = apv(ys[:], cp * 1560,
                          [[520, 3], [4, 3], [1, 512]])
                o = apv(P[:], 0,
                        [[1536, 3], [512, 3], [1, 512]])
                nc.vector.tensor_tensor(o, in0, in1, mybir.AluOpType.mult)
                for t in range(NMM):
                    r = mm % 16
                    b, q = r // 4, r % 4
                    if r == 0:
                        ps = pspool.tile([128, 2048], f32)
                    nc.tensor.matmul(ps[32 * q:32 * (q + 1),
                                        512 * b:512 * (b + 1)], ones[:],
                                     P[:, 512 * t:512 * (t + 1)],
                                     start=True, stop=True,
                                     tile_position=(0, 32 * q))
                    if r == 15:
                        stage = spool.tile([128, 2048], bf16)
                        nc.scalar.copy(stage[:], ps[:])
                        src = stage[:].copy()
                        src.ap = VecI64Pair([[8 * 2048, 16], [1, 2048]])
                        nc.gpsimd.dma_start(out[mm // 16], src)
                    mm += 1

    nc.finalize()
    return nc


def _get_program():
    if "nc" not in _PROG_CACHE:
        _PROG_CACHE["nc"] = _build_program()
    return _PROG_CACHE["nc"]


def _out_perm():
    """col (c', kw, kh, h'm4) -> flat out idx k2*16384 + h2*128 + w2."""
    if "perm" in _PROG_CACHE:
        return _PROG_CACHE["perm"]
    cp, kw, kh, hm = np.meshgrid(
        np.array(CP_ORDER), np.arange(3), np.arange(3), np.arange(512),
        indexing='ij')
    hp, m4 = hm // 4, hm % 4
    n = 9 * cp + 3 * kh + kw
    k2, m = n // 32, n % 32
    h2 = 4 * m + hp // 32
    w2 = 4 * (hp % 32) + m4
    perm = (k2 * 16384 + h2 * 128 + w2).reshape(-1)
    _PROG_CACHE["perm"] = perm
    return perm


def kernel(x: np.ndarray, y: np.ndarray) -> np.ndarray:
    import ml_dtypes
    from concourse.bass_utils import run_bass_kernel_spmd

    bf = ml_dtypes.bfloat16
    x = np.ascontiguousarray(np.asarray(x, dtype=np.float32))
    y = np.ascontiguousarray(np.asarray(y, dtype=np.float32))
    B, C_, D, H_, W_ = x.shape
    assert (B, C_, D, H_, W_) == (1, 32, 32, 128, 128)

    # depth-shifted, H/W-padded y (fp32, cast after gather)
    y_sp = np.zeros((D, C_, 130, 130), np.float32)
    y_sp[1:, :, 1:129, 1:129] = y[0].transpose(1, 0, 2, 3)[:-1]
    x_d = x[0].transpose(1, 0, 2, 3)  # [d, c, h, w]

    # x slab: [d, i, m, 512] = x[i, d, 4*(m%32) + col//128, col%128]
    ms = np.arange(MX) % 32
    xq = x_d.reshape(D, C_, 32, 512)                        # d i m32 col
    xq = np.ascontiguousarray(xq[:, :, ms]).astype(bf)      # d i m col

    # y slab: [d, i, c', kw, r, m4] = y_sp[d, c', r, 32*m4 + kw + i]
    i_ar = np.arange(32)[:, None, None]
    kw_ar = np.arange(3)[None, :, None]
    m4_ar = np.arange(4)[None, None, :]
    w_idx = 32 * m4_ar + kw_ar + i_ar     # [i, kw, m4]
    g = y_sp[:, :, :, w_idx]              # d c' r i kw m4
    yq = np.ascontiguousarray(g.transpose(0, 3, 1, 4, 2, 5)).astype(bf)

    ones_np = np.zeros((128, 32), bf)
    for m in range(32):
        g = m // 8
        ones_np[32 * g:32 * (g + 1), m] = 1

    nc = _get_program()
    in_maps = [
        {"xin": xq[4 * j:4 * j + 4].reshape(128, MX * 512),
         "yin": yq[4 * j:4 * j + 4].reshape(128, NBLK * 1560),
         "ones": ones_np}
        for j in range(N_CORES)
    ]
    res = run_bass_kernel_spmd(nc, in_maps, core_ids=list(range(N_CORES)),
                               trace=_RUN_OPTS["trace"])
    _LAST_RESULT["res"] = res

    perm = _out_perm()
    slabs = []
    for j in range(N_CORES):
        a = np.asarray(res.results[j]["out"]).astype(np.float32)
        a = a.reshape(NTILE, 4, 4, 4, 512)                      # t q g b n
        dec = a.transpose(2, 0, 3, 1, 4).reshape(4, 9 * 16384)  # g, colstream
        oc = np.empty((4, 9 * 16384), np.float32)
        oc[:, perm] = dec
        slabs.append(oc.reshape(4, 9, H, W))
    o = np.concatenate(slabs, axis=0)       # [32, 9, 128, 128]
    o = o.transpose(1, 0, 2, 3)             # [9, 32, 128, 128]
    o = np.where(o >= 0, o, 0.2 * o).astype(np.float32)
    return o[None]


# revision 32
# speedup vs baseline: 1.4138x; 1.0112x over previous
"""Trainium2 Bass kernel for nn_CorrTorch_unfold (B=1, C=32, D=32, H=W=128).

Reference math (incl. its raw-reshape scramble): with
F = k2*16384 + h2*128 + w2 and (c', k', G) = unravel(F, [32, 9, 512]),
kh' = k'//3, kw' = k'%3, h' = G//4, m4 = G%4:
  out[0,k2,d,h2,w2] = leaky_relu( sum_i x[i,d,h2,w2]
                                  * y_pad[c',d,h'+kh',32*m4+kw'+i] )
Equivalently, for n = 9c'+k': k2 = n//32, m = n%32, h2 = 4m + h'//32,
w2 = 4*(h'%32) + m4  (y_pad = y shifted one slice in depth, padded 1 in
H/W). The 32-term dot runs over x channels i paired with a contiguous
32-wide w-strip of y_pad.

v3 design (products-on-DVE + reduce-on-PE):
  Partition dim packs (d_local, i) = 4*32 = 128. DVE computes bf16
  products with the i-pairing baked into a host-interleaved y layout:
  Y_kw[(d,i), c'*520 + r*4 + m4] = y_pad[c', r, 32*m4 + kw + i].
  x stays in natural (h,w) layout, replicated into 47 "m-slots"
  (slot m = rows 4*(m%32)..+4) so the mod-32 slot walk m = n%32 becomes
  affine inside each TT. One TT per (c'-pair, kw') covers (c4, kh',
  h'*m4) = 2x3x512 free elems at 0.5 cyc/elem (2x_1p bf16 mode); a few
  TTs run on the otherwise-idle GpSimd engine to offload the DVE.
  The idle PE reduces over i: lhsT ones [128,32] sums each 32-partition
  group (depth groups duplicated 8x to fill full PSUM quadrants); 16
  matmuls fill a [128,2048] PSUM tile; ACT copies it to SBUF (bf16);
  one stride-8-partition DMA per tile extracts the 16 distinct rows.
  Blocks are processed in ascending x-slot order and loads are chunked
  so compute starts ~9us in. Leaky-relu + unscramble happen on host.

Sharding: D=32 depth slices, 4 per core across 8 cores.
"""
import numpy as np

_PROG_CACHE = {}
_RUN_OPTS = {"trace": False}
_LAST_RESULT = {}

D_LOC = 4
N_CORES = 8
C = 32
H = W = 128
MX = 40            # x m-slots (31 + 8 max walk: kw + 3*kh)
YCOLS = 32 * 130 * 4   # 16640 per kw slab
NBLK = 32          # one block per c'
PCOLS = 3 * 3 * 512    # 4608 product cols per c' tile (kw, kh, h'm4)
NMM = PCOLS // 512     # 9 matmuls per c'
TOTMM = NBLK * NMM     # 288
NTILE = TOTMM // 16    # 18 psum tiles -> out dumps

# process c' so the x slot window [m0, m0+8] (m0 = 9c'%32) advances by
# one slot per step: c' = 25t mod 32 gives m0 = t. Loads then stream
# just ahead of compute (y per-c' chunks, x in 5 slot-range chunks).
CP_ORDER = [(25 * t) % 32 for t in range(NBLK)]
X_CHUNKS = [(0, 9), (9, 17), (17, 25), (25, 33), (33, 40)]


def _build_program():
    import concourse.bacc as bacc
    import concourse.mybir as mybir
    from concourse.tile import TileContext
    from bass_rust import VecI64Pair

    f32 = mybir.dt.float32
    bf16 = mybir.dt.bfloat16

    def apv(base_ap, offset, dims):
        a = base_ap.copy()
        part = list(a.ap[0])
        a.ap = VecI64Pair([part] + [list(d) for d in dims])
        a.offset = a.offset + offset
        return a

    nc = bacc.Bacc()
    x_in = nc.dram_tensor("xin", [128, MX * 512], bf16, kind="ExternalInput")
    y_in = nc.dram_tensor("yin", [128, NBLK * 1560], bf16,
                          kind="ExternalInput")
    ones_in = nc.dram_tensor("ones", [128, 32], bf16, kind="ExternalInput")
    out = nc.dram_tensor("out", [NTILE, 16, 2048], bf16,
                         kind="ExternalOutput")

    with TileContext(nc) as tc:
        with tc.tile_pool(name="const", bufs=1) as cpool, \
             tc.tile_pool(name="p", bufs=4) as ppool, \
             tc.tile_pool(name="st", bufs=2) as spool, \
             tc.tile_pool(name="ps", bufs=2, space="PSUM") as pspool:

            ones = cpool.tile([128, 32], bf16)
            ys = cpool.tile([128, NBLK * 1560], bf16)  # kw inner per c'
            xs = cpool.tile([128, MX * 512], bf16)

            def yload(cp):
                nc.sync.dma_start(ys[:, cp * 1560:(cp + 1) * 1560],
                                  y_in[:, cp * 1560:(cp + 1) * 1560])

            def xload(xi):
                s0, s1 = X_CHUNKS[xi]
                nc.sync.dma_start(xs[:, s0 * 512:s1 * 512],
                                  x_in[:, s0 * 512:s1 * 512])

            # loads stream just ahead of the CP_ORDER compute sequence
            xload(0)
            yload(CP_ORDER[0])
            nc.sync.dma_start(ones[:], ones_in[:])
            xload(1)
            for t in range(1, 8):
                yload(CP_ORDER[t])
            xload(2)
            for t in range(8, 16):
                yload(CP_ORDER[t])
            xload(3)
            for t in range(16, 24):
                yload(CP_ORDER[t])
            xload(4)
            for t in range(24, 32):
                yload(CP_ORDER[t])

            mm = 0
            ps = None
            for cp in CP_ORDER:
                P = ppool.tile([128, PCOLS], bf16)
                m0 = (9 * cp) % 32
                in0 = apv(xs[:], m0 * 512,
                          [[512, 3], [3 * 512, 3], [1, 512]])
                in1 = apv(ys[:], cp * 1560,
                          [[520, 3], [4, 3], [1, 512]])
                o = apv(P[:], 0,
                        [[1536, 3], [512, 3], [1, 512]])
                nc.vector.tensor_tensor(o, in0, in1, mybir.AluOpType.mult)
                for t in range(NMM):
                    r = mm % 16
                    b, q = r // 4, r % 4
                    if r == 0:
                        ps = pspool.tile([128, 2048], f32)
                    nc.tensor.matmul(ps[32 * q:32 * (q + 1),
                                        512 * b:512 * (b + 1)], ones[:],
                                     P[:, 512 * t:512 * (t + 1)],
                                     start=True, stop=True,
                                     tile_position=(0, 32 * q))
                    if r == 15:
                        stage = spool.tile([128, 2048], bf16)
                        if mm == TOTMM - 1:
                            # pipeline the final tile's copy with its matmuls
                            for bb in range(4):
                                nc.scalar.copy(
                                    stage[:, 512 * bb:512 * (bb + 1)],
                                    ps[:, 512 * bb:512 * (bb + 1)])
                        else:
                            nc.scalar.copy(stage[:], ps[:])
                        src = stage[:].copy()
                        src.ap = VecI64Pair([[8 * 2048, 16], [1, 2048]])
                        nc.gpsimd.dma_start(out[mm // 16], src)
                    mm += 1

    nc.finalize()
    return nc


def _get_program():
    if "nc" not in _PROG_CACHE:
        _PROG_CACHE["nc"] = _build_program()
    return _PROG_CACHE["nc"]


def _out_perm():
    """col (c', kw, kh, h'm4) -> flat out idx k2*16384 + h2*128 + w2."""
    if "perm" in _PROG_CACHE:
        return _PROG_CACHE["perm"]
    cp, kw, kh, hm = np.meshgrid(
        np.array(CP_ORDER), np.arange(3), np.arange(3), np.arange(512),
        indexing='ij')
    hp, m4 = hm // 4, hm % 4
    n = 9 * cp + 3 * kh + kw
    k2, m = n // 32, n % 32
    h2 = 4 * m + hp // 32
    w2 = 4 * (hp % 32) + m4
    perm = (k2 * 16384 + h2 * 128 + w2).reshape(-1)
    _PROG_CACHE["perm"] = perm
    return perm


def kernel(x: np.ndarray, y: np.ndarray) -> np.ndarray:
    import ml_dtypes
    from concourse.bass_utils import run_bass_kernel_spmd

    bf = ml_dtypes.bfloat16
    x = np.ascontiguousarray(np.asarray(x, dtype=np.float32))
    y = np.ascontiguousarray(np.asarray(y, dtype=np.float32))
    B, C_, D, H_, W_ = x.shape
    assert (B, C_, D, H_, W_) == (1, 32, 32, 128, 128)

    # depth-shifted, H/W-padded y (fp32, cast after gather)
    y_sp = np.zeros((D, C_, 130, 130), np.float32)
    y_sp[1:, :, 1:129, 1:129] = y[0].transpose(1, 0, 2, 3)[:-1]
    x_d = x[0].transpose(1, 0, 2, 3)  # [d, c, h, w]

    # x slab: [d, i, m, 512] = x[i, d, 4*(m%32) + col//128, col%128]
    ms = np.arange(MX) % 32
    xq = x_d.reshape(D, C_, 32, 512)                        # d i m32 col
    xq = np.ascontiguousarray(xq[:, :, ms]).astype(bf)      # d i m col

    # y slab: [d, i, c', kw, r, m4] = y_sp[d, c', r, 32*m4 + kw + i]
    i_ar = np.arange(32)[:, None, None]
    kw_ar = np.arange(3)[None, :, None]
    m4_ar = np.arange(4)[None, None, :]
    w_idx = 32 * m4_ar + kw_ar + i_ar     # [i, kw, m4]
    g = y_sp[:, :, :, w_idx]              # d c' r i kw m4
    yq = np.ascontiguousarray(g.transpose(0, 3, 1, 4, 2, 5)).astype(bf)

    ones_np = np.zeros((128, 32), bf)
    for m in range(32):
        g = m // 8
        ones_np[32 * g:32 * (g + 1), m] = 1

    nc = _get_program()
    in_maps = [
        {"xin": xq[4 * j:4 * j + 4].reshape(128, MX * 512),
         "yin": yq[4 * j:4 * j + 4].reshape(128, NBLK * 1560),
         "ones": ones_np}
        for j in range(N_CORES)
    ]
    res = run_bass_kernel_spmd(nc, in_maps, core_ids=list(range(N_CORES)),
                               trace=_RUN_OPTS["trace"])
    _LAST_RESULT["res"] = res

    perm = _out_perm()
    slabs = []
    for j in range(N_CORES):
        a = np.asarray(res.results[j]["out"]).astype(np.float32)
        a = a.reshape(NTILE, 4, 4, 4, 512)                      # t q g b n
        dec = a.transpose(2, 0, 3, 1, 4).reshape(4, 9 * 16384)  # g, colstream
        oc = np.empty((4, 9 * 16384), np.float32)
        oc[:, perm] = dec
        slabs.append(oc.reshape(4, 9, H, W))
    o = np.concatenate(slabs, axis=0)       # [32, 9, 128, 128]
    o = o.transpose(1, 0, 2, 3)             # [9, 32, 128, 128]
    o = np.where(o >= 0, o, 0.2 * o).astype(np.float32)
    return o[None]


# revision 35
# speedup vs baseline: 1.4150x; 1.0009x over previous
"""Trainium2 Bass kernel for nn_CorrTorch_unfold (B=1, C=32, D=32, H=W=128).

Reference math (incl. its raw-reshape scramble): with
F = k2*16384 + h2*128 + w2 and (c', k', G) = unravel(F, [32, 9, 512]),
kh' = k'//3, kw' = k'%3, h' = G//4, m4 = G%4:
  out[0,k2,d,h2,w2] = leaky_relu( sum_i x[i,d,h2,w2]
                                  * y_pad[c',d,h'+kh',32*m4+kw'+i] )
Equivalently, for n = 9c'+k': k2 = n//32, m = n%32, h2 = 4m + h'//32,
w2 = 4*(h'%32) + m4  (y_pad = y shifted one slice in depth, padded 1 in
H/W). The 32-term dot runs over x channels i paired with a contiguous
32-wide w-strip of y_pad.

v3 design (products-on-DVE + reduce-on-PE):
  Partition dim packs (d_local, i) = 4*32 = 128. DVE computes bf16
  products with the i-pairing baked into a host-interleaved y layout:
  Y_kw[(d,i), c'*520 + r*4 + m4] = y_pad[c', r, 32*m4 + kw + i].
  x stays in natural (h,w) layout, replicated into 47 "m-slots"
  (slot m = rows 4*(m%32)..+4) so the mod-32 slot walk m = n%32 becomes
  affine inside each TT. One TT per (c'-pair, kw') covers (c4, kh',
  h'*m4) = 2x3x512 free elems at 0.5 cyc/elem (2x_1p bf16 mode); a few
  TTs run on the otherwise-idle GpSimd engine to offload the DVE.
  The idle PE reduces over i: lhsT ones [128,32] sums each 32-partition
  group (depth groups duplicated 8x to fill full PSUM quadrants); 16
  matmuls fill a [128,2048] PSUM tile; ACT copies it to SBUF (bf16);
  one stride-8-partition DMA per tile extracts the 16 distinct rows.
  Blocks are processed in ascending x-slot order and loads are chunked
  so compute starts ~9us in. Leaky-relu + unscramble happen on host.

Sharding: D=32 depth slices, 4 per core across 8 cores.
"""
import numpy as np

_PROG_CACHE = {}
_RUN_OPTS = {"trace": False}
_LAST_RESULT = {}

D_LOC = 4
N_CORES = 8
C = 32
H = W = 128
MX = 40            # x m-slots (31 + 8 max walk: kw + 3*kh)
YCOLS = 32 * 130 * 4   # 16640 per kw slab
NBLK = 32          # one block per c'
PCOLS = 3 * 3 * 512    # 4608 product cols per c' tile (kw, kh, h'm4)
NMM = PCOLS // 512     # 9 matmuls per c'
TOTMM = NBLK * NMM     # 288
NTILE = TOTMM // 16    # 18 psum tiles -> out dumps

# process c' so the x slot window [m0, m0+8] (m0 = 9c'%32) advances by
# one slot per step: c' = 25t mod 32 gives m0 = t. Loads then stream
# just ahead of compute (y per-c' chunks, x in 5 slot-range chunks).
CP_ORDER = [(25 * t) % 32 for t in range(NBLK)]
X_CHUNKS = [(0, 9), (9, 17), (17, 25), (25, 33), (33, 40)]


def _build_program():
    import concourse.bacc as bacc
    import concourse.mybir as mybir
    from concourse.tile import TileContext
    from bass_rust import VecI64Pair

    f32 = mybir.dt.float32
    bf16 = mybir.dt.bfloat16

    def apv(base_ap, offset, dims):
        a = base_ap.copy()
        part = list(a.ap[0])
        a.ap = VecI64Pair([part] + [list(d) for d in dims])
        a.offset = a.offset + offset
        return a

    nc = bacc.Bacc()
    x_in = nc.dram_tensor("xin", [128, MX * 512], bf16, kind="ExternalInput")
    y_in = nc.dram_tensor("yin", [128, NBLK * 1560], bf16,
                          kind="ExternalInput")
    ones_in = nc.dram_tensor("ones", [128, 32], bf16, kind="ExternalInput")
    out = nc.dram_tensor("out", [NTILE, 16, 2048], bf16,
                         kind="ExternalOutput")

    with TileContext(nc) as tc:
        with tc.tile_pool(name="const", bufs=1) as cpool, \
             tc.tile_pool(name="p", bufs=6) as ppool, \
             tc.tile_pool(name="st", bufs=2) as spool, \
             tc.tile_pool(name="ps", bufs=2, space="PSUM") as pspool:

            ones = cpool.tile([128, 32], bf16)
            ys = cpool.tile([128, NBLK * 1560], bf16)  # kw inner per c'
            xs = cpool.tile([128, MX * 512], bf16)

            def yload(cp):
                nc.sync.dma_start(ys[:, cp * 1560:(cp + 1) * 1560],
                                  y_in[:, cp * 1560:(cp + 1) * 1560])

            def xload(xi):
                s0, s1 = X_CHUNKS[xi]
                nc.sync.dma_start(xs[:, s0 * 512:s1 * 512],
                                  x_in[:, s0 * 512:s1 * 512])

            # loads stream just ahead of the CP_ORDER compute sequence
            xload(0)
            yload(CP_ORDER[0])
            nc.sync.dma_start(ones[:], ones_in[:])
            xload(1)
            for t in range(1, 8):
                yload(CP_ORDER[t])
            xload(2)
            for t in range(8, 16):
                yload(CP_ORDER[t])
            xload(3)
            for t in range(16, 24):
                yload(CP_ORDER[t])
            xload(4)
            for t in range(24, 32):
                yload(CP_ORDER[t])

            mm = 0
            ps = None
            for cp in CP_ORDER:
                P = ppool.tile([128, PCOLS], bf16)
                m0 = (9 * cp) % 32
                in0 = apv(xs[:], m0 * 512,
                          [[512, 3], [3 * 512, 3], [1, 512]])
                in1 = apv(ys[:], cp * 1560,
                          [[520, 3], [4, 3], [1, 512]])
                o = apv(P[:], 0,
                        [[1536, 3], [512, 3], [1, 512]])
                nc.vector.tensor_tensor(o, in0, in1, mybir.AluOpType.mult)
                for t in range(NMM):
                    r = mm % 16
                    b, q = r // 4, r % 4
                    if r == 0:
                        ps = pspool.tile([128, 2048], f32)
                    nc.tensor.matmul(ps[32 * q:32 * (q + 1),
                                        512 * b:512 * (b + 1)], ones[:],
                                     P[:, 512 * t:512 * (t + 1)],
                                     start=True, stop=True,
                                     tile_position=(0, 32 * q))
                    if mm >= TOTMM - 32:
                        # tail: copy+DMA each 512-col slice as its 4
                        # quadrant matmuls finish, pipelining the drain
                        if r == 3:
                            stage = spool.tile([128, 2048], bf16)
                        if q == 3:
                            nc.scalar.copy(
                                stage[:, 512 * b:512 * (b + 1)],
                                ps[:, 512 * b:512 * (b + 1)])
                            src = stage[:].copy()
                            src.ap = VecI64Pair([[8 * 2048, 16], [1, 512]])
                            src.offset = src.offset + 512 * b
                            nc.gpsimd.dma_start(
                                out[mm // 16, :, 512 * b:512 * (b + 1)],
                                src)
                    elif r == 15:
                        stage = spool.tile([128, 2048], bf16)
                        nc.scalar.copy(stage[:], ps[:])
                        src = stage[:].copy()
                        src.ap = VecI64Pair([[8 * 2048, 16], [1, 2048]])
                        nc.gpsimd.dma_start(out[mm // 16], src)
                    mm += 1

    nc.finalize()
    return nc


def _get_program():
    if "nc" not in _PROG_CACHE:
        _PROG_CACHE["nc"] = _build_program()
    return _PROG_CACHE["nc"]


def _out_perm():
    """col (c', kw, kh, h'm4) -> flat out idx k2*16384 + h2*128 + w2."""
    if "perm" in _PROG_CACHE:
        return _PROG_CACHE["perm"]
    cp, kw, kh, hm = np.meshgrid(
        np.array(CP_ORDER), np.arange(3), np.arange(3), np.arange(512),
        indexing='ij')
    hp, m4 = hm // 4, hm % 4
    n = 9 * cp + 3 * kh + kw
    k2, m = n // 32, n % 32
    h2 = 4 * m + hp // 32
    w2 = 4 * (hp % 32) + m4
    perm = (k2 * 16384 + h2 * 128 + w2).reshape(-1)
    _PROG_CACHE["perm"] = perm
    return perm


def kernel(x: np.ndarray, y: np.ndarray) -> np.ndarray:
    import ml_dtypes
    from concourse.bass_utils import run_bass_kernel_spmd

    bf = ml_dtypes.bfloat16
    x = np.ascontiguousarray(np.asarray(x, dtype=np.float32))
    y = np.ascontiguousarray(np.asarray(y, dtype=np.float32))
    B, C_, D, H_, W_ = x.shape
    assert (B, C_, D, H_, W_) == (1, 32, 32, 128, 128)

    # depth-shifted, H/W-padded y (fp32, cast after gather)
    y_sp = np.zeros((D, C_, 130, 130), np.float32)
    y_sp[1:, :, 1:129, 1:129] = y[0].transpose(1, 0, 2, 3)[:-1]
    x_d = x[0].transpose(1, 0, 2, 3)  # [d, c, h, w]

    # x slab: [d, i, m, 512] = x[i, d, 4*(m%32) + col//128, col%128]
    ms = np.arange(MX) % 32
    xq = x_d.reshape(D, C_, 32, 512)                        # d i m32 col
    xq = np.ascontiguousarray(xq[:, :, ms]).astype(bf)      # d i m col

    # y slab: [d, i, c', kw, r, m4] = y_sp[d, c', r, 32*m4 + kw + i]
    i_ar = np.arange(32)[:, None, None]
    kw_ar = np.arange(3)[None, :, None]
    m4_ar = np.arange(4)[None, None, :]
    w_idx = 32 * m4_ar + kw_ar + i_ar     # [i, kw, m4]
    g = y_sp[:, :, :, w_idx]              # d c' r i kw m4
    yq = np.ascontiguousarray(g.transpose(0, 3, 1, 4, 2, 5)).astype(bf)

    ones_np = np.zeros((128, 32), bf)
    for m in range(32):
        g = m // 8
        ones_np[32 * g:32 * (g + 1), m] = 1

    nc = _get_program()
    in_maps = [
        {"xin": xq[4 * j:4 * j + 4].reshape(128, MX * 512),
         "yin": yq[4 * j:4 * j + 4].reshape(128, NBLK * 1560),
         "ones": ones_np}
        for j in range(N_CORES)
    ]
    res = run_bass_kernel_spmd(nc, in_maps, core_ids=list(range(N_CORES)),
                               trace=_RUN_OPTS["trace"])
    _LAST_RESULT["res"] = res

    perm = _out_perm()
    slabs = []
    for j in range(N_CORES):
        a = np.asarray(res.results[j]["out"]).astype(np.float32)
        a = a.reshape(NTILE, 4, 4, 4, 512)                      # t q g b n
        dec = a.transpose(2, 0, 3, 1, 4).reshape(4, 9 * 16384)  # g, colstream
        oc = np.empty((4, 9 * 16384), np.float32)
        oc[:, perm] = dec
        slabs.append(oc.reshape(4, 9, H, W))
    o = np.concatenate(slabs, axis=0)       # [32, 9, 128, 128]
    o = o.transpose(1, 0, 2, 3)             # [9, 32, 128, 128]
    o = np.where(o >= 0, o, 0.2 * o).astype(np.float32)
    return o[None]
